# revision 63
# baseline (speedup 1.0000x reference)
"""Trainium2 Bass kernel for GrokAttention (S=1024, H=64, KVH=8, D=128, HID=8192).

Sharding: tensor-parallel over heads across 8 cores. Core c owns Q heads
[8c, 8c+8) and KV head c (GQA n_rep=8 maps KV head c exactly to those Q
heads). Each core computes a partial output out_c = attn_c @ Wo[rows of
core c]; the full output is the sum of the 8 partials (done on host at
gather time).

On-device layout is "transposed": qT/kT/vT are [head_dim, seq] so that
attention scores are computed as scoresT[s2, s1] with the 128-long head_dim
as the PE contraction dim. Softmax runs without max subtraction (logits are
tanh-capped to +-30 so exp cannot overflow); causal masking multiplies exp
by a 0/1 pattern; the denominator is a DVE tree-sum over key tiles followed
by a single ones-vector matmul per chunk, and 1/denom is computed after a
broadcast matmul with reciprocal_approx_fast.

All matmuls are bf16 x bf16 -> fp32 PSUM (fp8 was evaluated and rejected:
a straight cast is ~3.7e-2 error, and a hi+lo split needs 3 products per
k-tile vs DoubleRow's 2-per-instruction, a net 1.5x slowdown).

Schedule highlights (all found by reading neuron-profile traces):
- The DMA queue is FIFO and all engines execute in order, so K/V/q0
  weight slices are interleaved just ahead of the hs slice they pair
  with, and the three projections chase the 16 MB hs stream as it lands
  (the naive order stalls the PE ~50us).
- Per head: scores+exp+denominator-tree, then the NEXT head's projection,
  then softmax finish; the scalar exp chain (~11us) and the DVE adds hide
  under the 27us projection. The last head overlaps its softmax with the
  first output-projection chunk instead.
- Causal masking is ragged: key-tile t2 only computes score/exp/ov
  columns s1 >= t2*128 (right-aligned widths 512/384/256/128), -25% of
  scores/exp/ov work vs 512-wide chunks; diagonal 128-blocks multiply one
  shared triangular 0/1 pattern.
- The softmax reciprocal is computed on the [1,512] row and broadcast by
  a bf16 ones-matmul: an fp32 broadcast matmul is 8x slower and its
  fp32 LDWEIGHTS trips the HAM half-clock throttle.
- The output projection keeps Wo stationary ([e,s] output layout, host
  transposes back) so each LDWEIGHTS covers 2x512 moving columns; Wo
  streams through the then-dead wk_sb buffer as an 8-deep ring; the
  partial output is stored bf16 (halves store traffic).
- All weights are host-prearranged so every tile DMA is one contiguous
  per-partition run (2KB+ DMA packets instead of 256B).
"""

import sys
from contextlib import ExitStack

import numpy as np

for _p in ("/opt/trn_rl_repo",):
    if _p not in sys.path:
        sys.path.insert(0, _p)

import ml_dtypes
import concourse.bass as bass
import concourse.tile as tile
from concourse import bacc, mybir
from concourse.bass_utils import run_bass_kernel_spmd

F32 = mybir.dt.float32
BF16 = mybir.dt.bfloat16
BF = ml_dtypes.bfloat16

B, S, H, KVH, D = 1, 1024, 64, 8, 128
HID = H * D  # 8192
NCORES = 8
NQ = H // NCORES          # 8 q heads per core
QW = NQ * D               # 1024 q columns per core
ROPE_THETA = 208533496.0
LOGIT_CAP = 30.0
SCALE = 1.0 / float(np.sqrt(D))

NCH = HID // 128          # 64 hid chunks
SC = 512                  # seq chunk (psum-bank free dim)
NSC = S // SC             # 2
NEP = HID // 256          # 32 wo e-pairs (2 x 128 e-cols per tile)


def build_nc():
    nc = bacc.Bacc()
    hsT = nc.declare_dram_parameter("hsT", [HID, S], BF16, isOutput=False)
    # weights host-prearranged and flattened 2D so every tile DMA is one
    # contiguous per-partition run (big DMA packets):
    # wq [p, head*chunk*m], wk/wv [p, chunk*m], wo [p, e_chunk*hh*m]
    wq = nc.declare_dram_parameter("wq", [D, NQ * HID], BF16, isOutput=False)
    wk = nc.declare_dram_parameter("wk", [D, HID], BF16, isOutput=False)
    wv = nc.declare_dram_parameter("wv", [D, HID], BF16, isOutput=False)
    wo = nc.declare_dram_parameter("wo", [D, NCH * QW], BF16, isOutput=False)
    cosT = nc.declare_dram_parameter("cosT", [D, S], BF16, isOutput=False)
    sinT2 = nc.declare_dram_parameter("sinT2", [D, S], BF16, isOutput=False)
    masks = nc.declare_dram_parameter("masks", [D, D], BF16, isOutput=False)
    perm = nc.declare_dram_parameter("perm", [D, D], BF16, isOutput=False)
    ident = nc.declare_dram_parameter("ident", [D, D], BF16, isOutput=False)
    onesd = nc.declare_dram_parameter("onesd", [D, 1], BF16, isOutput=False)
    onesr = nc.declare_dram_parameter("onesr", [1, D], BF16, isOutput=False)
    outp = nc.declare_dram_parameter("outp", [HID, S], BF16, isOutput=True)

    with tile.TileContext(nc) as tc:
        with ExitStack() as ctx:
            build_kernel(ctx, tc, hsT, wq, wk, wv, wo, cosT, sinT2, masks,
                         perm, ident, onesd, onesr, outp)
    nc.compile()
    return nc


def build_kernel(ctx, tc, hsT, wq, wk, wv, wo, cosT, sinT2, masks, perm,
                 ident, onesd, onesr, outp):
    nc = tc.nc
    AF = mybir.ActivationFunctionType

    persist = ctx.enter_context(tc.tile_pool(name="persist", bufs=1))
    qpool = ctx.enter_context(tc.tile_pool(name="qpool", bufs=2))
    hspool = ctx.enter_context(tc.tile_pool(name="hspool", bufs=1))
    wstr = ctx.enter_context(tc.tile_pool(name="wstr", bufs=2))
    big = ctx.enter_context(tc.tile_pool(name="big", bufs=2))
    small = ctx.enter_context(tc.tile_pool(name="small", bufs=2))
    psum = ctx.enter_context(tc.tile_pool(name="psum", bufs=4, space="PSUM"))
    psum_dn = ctx.enter_context(tc.tile_pool(name="psum_dn", bufs=2, space="PSUM"))
    psum_tr = ctx.enter_context(tc.tile_pool(name="psum_tr", bufs=2, space="PSUM"))

    # ---- constants (small, land fast, ahead of the big streams) ----------
    cos_sb = persist.tile([D, S], BF16, tag="cos")
    sin_sb = persist.tile([D, S], BF16, tag="sin")
    mask_sb = persist.tile([D, D], BF16, tag="mask")
    perm_sb = persist.tile([D, D], BF16, tag="perm")
    ident_sb = persist.tile([D, D], BF16, tag="ident")
    ones_sb = persist.tile([D, 1], BF16, tag="ones")
    onesr_sb = persist.tile([1, D], BF16, tag="onesr")
    def load_consts():
        # deferred: queued mid hs-stream, well before first use (rope at
        # ~55us), so the startup-critical hs/weight bytes go first
        nc.sync.dma_start(cos_sb[:], cosT[:])
        nc.sync.dma_start(sin_sb[:], sinT2[:])
        nc.sync.dma_start(mask_sb[:], masks[:])
        nc.sync.dma_start(perm_sb[:], perm[:])
        nc.sync.dma_start(ident_sb[:], ident[:])
        nc.sync.dma_start(ones_sb[:], onesd[:])
        nc.sync.dma_start(onesr_sb[:], onesr[:])

    # persistent activations
    k_sb = persist.tile([128, S], BF16, tag="k_sb")
    v_sb = persist.tile([128, NQ, D], BF16, tag="vnat")   # v natural [s2-tile][s2_in, d]
    oT_sb = persist.tile([128, NQ, S], BF16, tag="oT")    # per-head o^T [d, s1]

    # Wk fully preloaded BEFORE the hs stream (FIFO dma queue). The K, V
    # and q0 projections then chase hs slices as they land; V/q0 weight
    # halves are interleaved into the stream just ahead of the hs slice
    # they are consumed with.
    wk_sb = persist.tile([128, HID], BF16, tag="wk")

    hsT_v = hsT.rearrange("(c p) s -> p c s", p=128)      # [128, 64, 1024]
    # first hs slice split fine (2+2+4 chunks) so the earliest matmuls
    # start after only 0.5 MB of hs has landed; all weight streams are
    # interleaved per-part just ahead of the hs slice they pair with
    hs_tiles, wv_tiles, wq0_tiles = {}, [], []
    for part in range(8):
        csl = slice(part * 8, (part + 1) * 8)
        wsl = slice(part * 8 * D, (part + 1) * 8 * D)
        nc.sync.dma_start(wk_sb[:, wsl], wk[:, wsl])
        wvt = big.tile([128, 8 * D], BF16, tag="wo", name=f"wv{part}")
        nc.sync.dma_start(wvt[:], wv[:, wsl])
        wv_tiles.append(wvt)
        wqt = wstr.tile([128, 8 * D], BF16, tag="w1", name=f"wq0_{part}")
        nc.sync.dma_start(wqt[:], wq[:, wsl])
        wq0_tiles.append(wqt)
        if part == 0:
            for g, (c0, w) in enumerate([(0, 2), (2, 2), (4, 4)]):
                t = hspool.tile([128, w, S], BF16, tag=f"hsg{g}", name=f"hsg{g}")
                nc.sync.dma_start(t[:], hsT_v[:, c0:c0 + w, :])
                for c in range(w):
                    hs_tiles[c0 + c] = (t, c)
        else:
            t = hspool.tile([128, 8, S], BF16, tag=f"hs{part}", name=f"hs{part}")
            nc.sync.dma_start(t[:], hsT_v[:, csl, :])
            for c in range(8):
                hs_tiles[part * 8 + c] = (t, c)
            if part == 3:
                load_consts()

    def hs_chunk(cc, sl):
        t, c = hs_tiles[cc]
        return t[:, c, sl]

    def mm_pair(outs, lhsT, rhss, start, stop):
        """Consecutive matmuls sharing one stationary operand: elide the
        redundant LDWEIGHTS on all but the first."""
        for i, (o, r) in enumerate(zip(outs, rhss)):
            inst = nc.tensor.matmul(o, lhsT, r, start=start, stop=stop)
            if i > 0:
                inst.ins.ldweights = False

    w_srcs = {j: wq[:, j * HID:(j + 1) * HID] for j in range(NQ)}

    def project(src_key, dst_sb):
        """dst_sb[128, S] (bf16) = (W_col^T @ hs) for one 128-wide column."""
        ps = [psum.tile([128, SC], F32, tag="mm512", name=f"pj{s}")
              for s in range(NSC)]
        for half in range(8):
            # alternate between the two rings -> effective prefetch depth 4
            pl, tg = (wstr, "w1") if half % 2 == 0 else (big, "wo")
            w_t = pl.tile([128, 8 * D], BF16, tag=tg)
            nc.sync.dma_start(
                w_t[:], w_srcs[src_key][:, half * 8 * D:(half + 1) * 8 * D])
            for c in range(NCH // 8):
                cc = half * 8 + c
                mm_pair([ps[s][:] for s in range(NSC)],
                        w_t[:, c * D:(c + 1) * D],
                        [hs_chunk(cc, slice(s * SC, (s + 1) * SC))
                         for s in range(NSC)],
                        start=(cc == 0), stop=(cc == NCH - 1))
        for s in range(NSC):
            # vector, not scalar: the scalar queue is ~13us deep with the
            # exp chain, and rope + next-head scores wait on this copy
            nc.vector.tensor_copy(dst_sb[:, s * SC:(s + 1) * SC], ps[s][:])

    def rope(src_sb):
        # in-place: src = src * cosT + (perm.T @ src) * sinT2
        shs = [psum_tr.tile([128, SC], F32, tag="shift", name=f"sh{s}")
               for s in range(NSC)]
        mm_pair([sh[:] for sh in shs], perm_sb[:],
                [src_sb[:, s * SC:(s + 1) * SC] for s in range(NSC)],
                start=True, stop=True)
        for s in range(NSC):
            sl = slice(s * SC, (s + 1) * SC)
            tmp = small.tile([128, SC], F32, tag="tanh")
            # cos-multiply first: it does not depend on the perm matmul, so
            # it drains from the DVE queue while the matmul is in flight
            nc.vector.tensor_mul(src_sb[:, sl], src_sb[:, sl], cos_sb[:, sl])
            nc.vector.tensor_mul(tmp[:], shs[s][:], sin_sb[:, sl])
            nc.vector.tensor_add(src_sb[:, sl], src_sb[:, sl], tmp[:])

    # Causal geometry: key tile t2 only attends queries s1 >= t2*128, so its
    # score/exp/ov tile within chunk ch is only w = 512 - max(0, t2*128 -
    # ch*512) columns wide (right-aligned). Diagonal 128-blocks (first 128
    # cols of each ragged tile in its own chunk) take a triangular mask.
    def tile_geom(ch):
        out = []
        for t2 in range(0, min(NQ, (ch + 1) * 4)):
            w = SC - max(0, t2 * D - ch * SC)
            out.append((t2, w, SC - w))
        return out

    EOFFS, _pos = {}, 0
    for _ch in range(NSC):
        for _t2, _w, _off in tile_geom(_ch):
            EOFFS[(_t2, _ch)] = (_pos, _w)
            _pos += _w
    EXPW = _pos  # 4608

    def scores_exp(j, qrope):
        # per chunk: score matmuls -> tanh/exp -> mask, then immediately the
        # chunk's denominator DVE tree, so the ch0 tree is not queued behind
        # the ch1 exp chain on the vector engine
        expT = big.tile([128, EXPW], BF16, tag="big8k", bufs=1,
                        name=f"expT{j}")
        accbs = []
        for ch in range(NSC):
            geom = tile_geom(ch)
            for t2, w, off in geom:
                sc_ps = psum.tile([128, SC], F32, tag="mm512", name="sc")
                nc.tensor.matmul(sc_ps[:, 0:w], k_sb[:, t2 * D:(t2 + 1) * D],
                                 qrope[:, ch * SC + off:(ch + 1) * SC],
                                 start=True, stop=True)
                tmp = small.tile([128, SC], F32, tag="tanh")
                nc.scalar.activation(tmp[:, 0:w], sc_ps[:, 0:w], AF.Tanh,
                                     scale=SCALE / LOGIT_CAP)
                es, _ = EOFFS[(t2, ch)]
                dst = expT[:, es:es + w]
                nc.scalar.activation(dst, tmp[:, 0:w], AF.Exp, scale=LOGIT_CAP)
                if t2 // 4 == ch:
                    nc.vector.tensor_mul(expT[:, es:es + D], expT[:, es:es + D],
                                         mask_sb[:])
            acc = small.tile([128, SC], F32, tag="tanh", name="dacc")
            es0, w0 = EOFFS[(geom[0][0], ch)]
            nc.vector.tensor_copy(acc[:], expT[:, es0:es0 + w0])
            for t2, w, off in geom[1:]:
                es, _ = EOFFS[(t2, ch)]
                nc.vector.tensor_add(acc[:, off:SC], acc[:, off:SC],
                                     expT[:, es:es + w])
            accb = small.tile([128, SC], BF16, tag="acb", name=f"accb{ch}")
            nc.vector.tensor_copy(accb[:], acc[:])
            accbs.append(accb)
        return expT, accbs

    def attn_finish(j, expT, accbs):
        for ch in range(NSC):
            sl = slice(ch * SC, (ch + 1) * SC)
            geom = tile_geom(ch)
            # ov first: its matmuls chase the exp chain tile by tile
            ov = psum.tile([128, SC], F32, tag="mm512", name="ovps")
            for i, (t2, w, off) in enumerate(geom):
                es, _ = EOFFS[(t2, ch)]
                nc.tensor.matmul(ov[:, off:SC], v_sb[:, t2, :],
                                 expT[:, es:es + w],
                                 start=(i == 0), stop=(i == len(geom) - 1),
                                 skip_group_check=True)
            dn = psum_dn.tile([1, SC], F32, tag="dn")
            nc.tensor.matmul(dn[:], ones_sb[:], accbs[ch][:],
                             start=True, stop=True)
            # reciprocal on the [1,512] row, then broadcast in bf16 (an fp32
            # broadcast matmul is 8x slower and trips the HAM throttle; a
            # gpsimd partition_broadcast was tried and is ~1us + sem hops,
            # stalling the DVE queue at the final multiply)
            dnr = small.tile([1, SC], F32, tag="rcx", name="dnr")
            nc.vector.reciprocal_approx_fast(out=dnr[:], in_=dn[:])
            dnrb = small.tile([1, SC], BF16, tag="rcx", name="dnrb")
            nc.scalar.copy(dnrb[:], dnr[:])
            rcb_ps = psum.tile([128, SC], F32, tag="mm512", name="rcbps")
            nc.tensor.matmul(rcb_ps[:], onesr_sb[:], dnrb[:],
                             start=True, stop=True)
            rcbs = small.tile([128, SC], BF16, tag="rcx", name="rcbs")
            nc.scalar.copy(rcbs[:], rcb_ps[:])
            nc.vector.tensor_mul(oT_sb[:, j, sl], ov[:], rcbs[:])

    # ---- fused K+V+q0 projection chasing the hs stream --------------------
    vT_sb = qpool.tile([128, S], BF16, tag="qh", name="vT")
    qrope = qpool.tile([128, S], BF16, tag="qh", name="q0")
    kps = [psum.tile([128, SC], F32, tag="mm512", name=f"kp{s}")
           for s in range(NSC)]
    vps = [psum.tile([128, SC], F32, tag="mm512", name=f"vp{s}")
           for s in range(NSC)]
    qps = [psum_tr.tile([128, SC], F32, tag="shift", name=f"qp{s}")
           for s in range(NSC)]
    for cc in range(NCH):
        part, c = cc >> 3, cc & 7
        hsp = [hs_chunk(cc, slice(s * SC, (s + 1) * SC)) for s in range(NSC)]
        st, sp = (cc == 0), (cc == NCH - 1)
        mm_pair([p[:] for p in kps], wk_sb[:, cc * D:(cc + 1) * D], hsp,
                start=st, stop=sp)
        mm_pair([p[:] for p in vps], wv_tiles[part][:, c * D:(c + 1) * D], hsp,
                start=st, stop=sp)
        mm_pair([p[:] for p in qps], wq0_tiles[part][:, c * D:(c + 1) * D], hsp,
                start=st, stop=sp)
    for s in range(NSC):
        sl = slice(s * SC, (s + 1) * SC)
        nc.scalar.copy(k_sb[:, sl], kps[s][:])
        nc.vector.tensor_copy(vT_sb[:, sl], vps[s][:])
        nc.vector.tensor_copy(qrope[:, sl], qps[s][:])
    rope(k_sb)
    for t2 in range(NQ):
        vt = psum_tr.tile([128, SC], BF16, tag="shift", name=f"vt{t2}")
        nc.tensor.transpose(vt[:, :D], vT_sb[:, t2 * D:(t2 + 1) * D],
                            ident_sb[:])
        nc.vector.tensor_copy(v_sb[:, t2, :], vt[:, :D])
    rope(qrope)

    # ---- output projection helpers ----------------------------------------
    # Wo chunks stream through the (dead after K projection) wk_sb buffer as
    # an 8-deep ring, so loads run many chunks ahead of the matmuls.
    def oproj_open(ec, pl, tag):
        base = (ec % 8) * QW
        wo_t = wk_sb[:, base:base + QW]
        nc.sync.dma_start(wo_t, wo[:, ec * QW:(ec + 1) * QW])
        ps = [pl.tile([128, SC], F32, tag=tag, name=f"op{ec}_{s}")
              for s in range(NSC)]
        for hh in range(NQ - 1):
            for s in range(NSC):
                nc.tensor.matmul(ps[s][:], wo_t[:, hh * D:(hh + 1) * D],
                                 oT_sb[:, hh, s * SC:(s + 1) * SC],
                                 start=(hh == 0), stop=False)
        return wo_t, ps

    def oproj_close(ec, wo_t, ps):
        for s in range(NSC):
            nc.tensor.matmul(ps[s][:], wo_t[:, (NQ - 1) * D:NQ * D],
                             oT_sb[:, NQ - 1, s * SC:(s + 1) * SC],
                             start=False, stop=True)
        for s in range(NSC):
            ot = small.tile([128, SC], BF16, tag="rcx", name="ot")
            nc.vector.tensor_copy(ot[:], ps[s][:])
            nc.sync.dma_start(
                outp[ec * 128:(ec + 1) * 128, s * SC:(s + 1) * SC], ot[:])

    # ---- fused Q projection + attention, software-pipelined ---------------
    # Emit head j+1's projection between head j's scores and its softmax
    # finish, so the in-order tensor engine never waits on the scalar/vector
    # exp chain. The last head overlaps its softmax with the first two
    # O-projection chunks' hh=0..6 accumulation instead.
    pre = []
    for j in range(NQ):
        expT, accbs = scores_exp(j, qrope)
        if j + 1 < NQ:
            qrope_next = qpool.tile([128, S], BF16, tag="qh", name=f"q{j + 1}")
            project(j + 1, qrope_next)
        else:
            pre.append((0, *oproj_open(0, psum_tr, "shift")))
        # rope before attn_finish: the perm matmul only waits on the DVE
        # q-copy (short queue), and the rope DVE muls then drain under
        # attn_finish's ov/dn/rcb tensor work, so the next head's scores
        # start without a vector-queue wait
        if j + 1 < NQ:
            rope(qrope_next)
        attn_finish(j, expT, accbs)
        if j + 1 < NQ:
            qrope = qrope_next

    # ---- output projection: out[e, s] = sum_hh wo[:, hh, e]^T @ oT[:, hh, s]
    for ec, wo_t, ps in pre:
        oproj_close(ec, wo_t, ps)
    for ec in range(1, NCH):
        wo_t, ps = oproj_open(ec, psum, "mm512")
        oproj_close(ec, wo_t, ps)


# --------------------------------------------------------------------------
# host side
# --------------------------------------------------------------------------

def _rope_tables(position_ids):
    pos = np.asarray(position_ids).reshape(-1).astype(np.int64)
    inv_freq = (1.0 / (ROPE_THETA ** (np.arange(0, D, 2, dtype=np.float32) / D))
                ).astype(np.float32)
    t = np.arange(S, dtype=np.float32)
    freqs = np.outer(t, inv_freq).astype(np.float32)       # (S, D/2)
    emb = np.concatenate((freqs, freqs), axis=-1)          # (S, D)
    cos = np.cos(emb).astype(np.float32)[pos]              # (S, D)
    sin = np.sin(emb).astype(np.float32)[pos]
    cosT = np.ascontiguousarray(cos.T)                     # (D, S)
    sinT = np.ascontiguousarray(sin.T)
    sinT2 = sinT.copy()
    sinT2[: D // 2] *= -1.0                                # rotate_half sign
    return cosT, sinT2


def _mask_patterns(attention_mask):
    # single diagonal-block pattern: allowed(s2 = p, s1 = j) for p, j < 128
    am = np.asarray(attention_mask)[0, 0]                  # (S_q, S_k)
    return np.ascontiguousarray((am[:D, :D].T > -0.5).astype(np.float32)).astype(BF)


_NC = None


def _get_nc():
    global _NC
    if _NC is None:
        _NC = build_nc()
    return _NC


def make_in_maps(hidden_states, Wq, Wk, Wv, Wo, attention_mask, position_ids):
    hsT = np.ascontiguousarray(
        np.asarray(hidden_states)[0].T.astype(np.float32)).astype(BF)
    cosT, sinT2 = _rope_tables(position_ids)
    masks = _mask_patterns(attention_mask)
    perm = np.zeros((D, D), dtype=np.float32)
    for d in range(D):
        perm[(d + 64) % 128, d] = 1.0
    perm = perm.astype(BF)
    ident = np.eye(D, dtype=np.float32).astype(BF)
    onesd = np.ones((D, 1), dtype=np.float32).astype(BF)
    Wq = np.asarray(Wq)
    Wk = np.asarray(Wk)
    Wv = np.asarray(Wv)
    Wo = np.asarray(Wo)
    in_maps = []
    for c in range(NCORES):
        # Wo rows of this core: [QW, HID] -> [hh, d, e] -> [d, e_chunk*hh*128]
        woc = Wo[c * QW:(c + 1) * QW, :].reshape(NQ, D, HID)
        woc = np.ascontiguousarray(
            woc.transpose(1, 2, 0).reshape(D, NCH, 128, NQ).transpose(0, 1, 3, 2)
        ).reshape(D, NCH * QW)
        # wq [8192, 1024] -> [p, head*chunk*m]; wk/wv [8192, 128] -> [p, chunk*m]
        wqc = np.ascontiguousarray(
            Wq[:, c * QW:(c + 1) * QW].reshape(NCH, D, NQ, D)
            .transpose(1, 2, 0, 3)).reshape(D, NQ * HID)
        wkc = np.ascontiguousarray(
            Wk[:, c * D:(c + 1) * D].reshape(NCH, D, D)
            .transpose(1, 0, 2)).reshape(D, HID)
        wvc = np.ascontiguousarray(
            Wv[:, c * D:(c + 1) * D].reshape(NCH, D, D)
            .transpose(1, 0, 2)).reshape(D, HID)
        in_maps.append({
            "hsT": hsT,
            "wq": wqc.astype(BF),
            "wk": wkc.astype(BF),
            "wv": wvc.astype(BF),
            "wo": woc.astype(BF),
            "cosT": cosT.astype(BF), "sinT2": sinT2.astype(BF), "masks": masks,
            "perm": perm, "ident": ident, "onesd": onesd,
            "onesr": np.ones((1, D), dtype=np.float32).astype(BF),
        })
    return in_maps


def kernel(hidden_states, Wq, Wk, Wv, Wo, attention_mask, position_ids,
           _trace=False):
    nc = _get_nc()
    in_maps = make_in_maps(hidden_states, Wq, Wk, Wv, Wo, attention_mask,
                           position_ids)
    res = run_bass_kernel_spmd(nc, in_maps, list(range(NCORES)), trace=_trace)
    out = np.zeros((HID, S), dtype=np.float32)
    for c in range(NCORES):
        out += res.results[c]["outp"].astype(np.float32)
    ret = np.ascontiguousarray(out.T).reshape(B, S, HID)
    if _trace:
        kernel.last_exec_time_ns = res.exec_time_ns
        kernel.last_results = res
    return ret


# revision 66
# speedup vs baseline: 1.0257x; 1.0257x over previous
"""Trainium2 Bass kernel for GrokAttention (S=1024, H=64, KVH=8, D=128, HID=8192).

Sharding: tensor-parallel over heads across 8 cores. Core c owns Q heads
[8c, 8c+8) and KV head c (GQA n_rep=8 maps KV head c exactly to those Q
heads). Each core computes a partial output out_c = attn_c @ Wo[rows of
core c]; the full output is the sum of the 8 partials (done on host at
gather time).

On-device layout is "transposed": qT/kT/vT are [head_dim, seq] so that
attention scores are computed as scoresT[s2, s1] with the 128-long head_dim
as the PE contraction dim. Softmax runs without max subtraction (logits are
tanh-capped to +-30 so exp cannot overflow); causal masking multiplies exp
by a 0/1 pattern; the denominator is a DVE tree-sum over key tiles followed
by a single ones-vector matmul per chunk, and 1/denom is computed after a
broadcast matmul with reciprocal_approx_fast.

All matmuls are bf16 x bf16 -> fp32 PSUM (fp8 was evaluated and rejected:
a straight cast is ~3.7e-2 error, and a hi+lo split needs 3 products per
k-tile vs DoubleRow's 2-per-instruction, a net 1.5x slowdown).

Schedule highlights (all found by reading neuron-profile traces):
- The DMA queue is FIFO and all engines execute in order, so K/V/q0
  weight slices are interleaved just ahead of the hs slice they pair
  with, and the three projections chase the 16 MB hs stream as it lands
  (the naive order stalls the PE ~50us).
- Per head: scores+exp+denominator-tree, then the NEXT head's projection,
  then softmax finish; the scalar exp chain (~11us) and the DVE adds hide
  under the 27us projection. The last head overlaps its softmax with the
  first output-projection chunk instead.
- Causal masking is ragged: key-tile t2 only computes score/exp/ov
  columns s1 >= t2*128 (right-aligned widths 512/384/256/128), -25% of
  scores/exp/ov work vs 512-wide chunks; diagonal 128-blocks multiply one
  shared triangular 0/1 pattern.
- The softmax reciprocal is computed on the [1,512] row and broadcast by
  a bf16 ones-matmul: an fp32 broadcast matmul is 8x slower and its
  fp32 LDWEIGHTS trips the HAM half-clock throttle.
- The output projection keeps Wo stationary ([e,s] output layout, host
  transposes back) so each LDWEIGHTS covers 2x512 moving columns; Wo
  streams through the then-dead wk_sb buffer as an 8-deep ring; the
  partial output is stored bf16 (halves store traffic).
- All weights are host-prearranged so every tile DMA is one contiguous
  per-partition run (2KB+ DMA packets instead of 256B).
"""

import sys
from contextlib import ExitStack

import numpy as np

for _p in ("/opt/trn_rl_repo",):
    if _p not in sys.path:
        sys.path.insert(0, _p)

import ml_dtypes
import concourse.bass as bass
import concourse.tile as tile
from concourse import bacc, mybir
from concourse.bass_utils import run_bass_kernel_spmd

F32 = mybir.dt.float32
BF16 = mybir.dt.bfloat16
BF = ml_dtypes.bfloat16

B, S, H, KVH, D = 1, 1024, 64, 8, 128
HID = H * D  # 8192
NCORES = 8
NQ = H // NCORES          # 8 q heads per core
QW = NQ * D               # 1024 q columns per core
ROPE_THETA = 208533496.0
LOGIT_CAP = 30.0
SCALE = 1.0 / float(np.sqrt(D))

NCH = HID // 128          # 64 hid chunks
SC = 512                  # seq chunk (psum-bank free dim)
NSC = S // SC             # 2
NEP = HID // 256          # 32 wo e-pairs (2 x 128 e-cols per tile)


def build_nc():
    nc = bacc.Bacc()
    hsT = nc.declare_dram_parameter("hsT", [HID, S], BF16, isOutput=False)
    # weights host-prearranged and flattened 2D so every tile DMA is one
    # contiguous per-partition run (big DMA packets):
    # wq [p, head*chunk*m], wk/wv [p, chunk*m], wo [p, e_chunk*hh*m]
    wq = nc.declare_dram_parameter("wq", [D, NQ * HID], BF16, isOutput=False)
    wk = nc.declare_dram_parameter("wk", [D, HID], BF16, isOutput=False)
    wv = nc.declare_dram_parameter("wv", [D, HID], BF16, isOutput=False)
    wo = nc.declare_dram_parameter("wo", [D, NCH * QW], BF16, isOutput=False)
    cosT = nc.declare_dram_parameter("cosT", [D, S], BF16, isOutput=False)
    sinT2 = nc.declare_dram_parameter("sinT2", [D, S], BF16, isOutput=False)
    masks = nc.declare_dram_parameter("masks", [D, D], BF16, isOutput=False)
    perm = nc.declare_dram_parameter("perm", [D, D], BF16, isOutput=False)
    ident = nc.declare_dram_parameter("ident", [D, D], BF16, isOutput=False)
    onesd = nc.declare_dram_parameter("onesd", [D, 1], BF16, isOutput=False)
    onesr = nc.declare_dram_parameter("onesr", [1, D], BF16, isOutput=False)
    outp = nc.declare_dram_parameter("outp", [HID, S], BF16, isOutput=True)

    with tile.TileContext(nc) as tc:
        with ExitStack() as ctx:
            build_kernel(ctx, tc, hsT, wq, wk, wv, wo, cosT, sinT2, masks,
                         perm, ident, onesd, onesr, outp)
    nc.compile()
    return nc


def build_kernel(ctx, tc, hsT, wq, wk, wv, wo, cosT, sinT2, masks, perm,
                 ident, onesd, onesr, outp):
    nc = tc.nc
    AF = mybir.ActivationFunctionType

    persist = ctx.enter_context(tc.tile_pool(name="persist", bufs=1))
    qpool = ctx.enter_context(tc.tile_pool(name="qpool", bufs=2))
    hspool = ctx.enter_context(tc.tile_pool(name="hspool", bufs=1))
    wstr = ctx.enter_context(tc.tile_pool(name="wstr", bufs=2))
    big = ctx.enter_context(tc.tile_pool(name="big", bufs=2))
    small = ctx.enter_context(tc.tile_pool(name="small", bufs=2))
    psum = ctx.enter_context(tc.tile_pool(name="psum", bufs=4, space="PSUM"))
    psum_dn = ctx.enter_context(tc.tile_pool(name="psum_dn", bufs=2, space="PSUM"))
    psum_tr = ctx.enter_context(tc.tile_pool(name="psum_tr", bufs=2, space="PSUM"))

    # ---- constants (small, land fast, ahead of the big streams) ----------
    cos_sb = persist.tile([D, S], BF16, tag="cos")
    sin_sb = persist.tile([D, S], BF16, tag="sin")
    mask_sb = persist.tile([D, D], BF16, tag="mask")
    perm_sb = persist.tile([D, D], BF16, tag="perm")
    ident_sb = persist.tile([D, D], BF16, tag="ident")
    ones_sb = persist.tile([D, 1], BF16, tag="ones")
    onesr_sb = persist.tile([1, D], BF16, tag="onesr")
    def load_consts():
        # deferred: queued mid hs-stream, well before first use (rope at
        # ~55us), so the startup-critical hs/weight bytes go first
        nc.sync.dma_start(cos_sb[:], cosT[:])
        nc.sync.dma_start(sin_sb[:], sinT2[:])
        nc.sync.dma_start(mask_sb[:], masks[:])
        nc.sync.dma_start(perm_sb[:], perm[:])
        nc.sync.dma_start(ident_sb[:], ident[:])
        nc.sync.dma_start(ones_sb[:], onesd[:])
        nc.sync.dma_start(onesr_sb[:], onesr[:])

    # persistent activations
    k_sb = persist.tile([128, S], BF16, tag="k_sb")
    v_sb = persist.tile([128, NQ, D], BF16, tag="vnat")   # v natural [s2-tile][s2_in, d]
    oT_sb = persist.tile([128, NQ, S], BF16, tag="oT")    # per-head o^T [d, s1]

    # Wk fully preloaded BEFORE the hs stream (FIFO dma queue). The K, V
    # and q0 projections then chase hs slices as they land; V/q0 weight
    # halves are interleaved into the stream just ahead of the hs slice
    # they are consumed with.
    wk_sb = persist.tile([128, HID], BF16, tag="wk")

    hsT_v = hsT.rearrange("(c p) s -> p c s", p=128)      # [128, 64, 1024]
    # first hs slice split fine (2+2+4 chunks) so the earliest matmuls
    # start after only 0.5 MB of hs has landed; all weight streams are
    # interleaved per-part just ahead of the hs slice they pair with
    hs_tiles, wv_tiles, wq0_tiles = {}, [], []
    for part in range(8):
        csl = slice(part * 8, (part + 1) * 8)
        wsl = slice(part * 8 * D, (part + 1) * 8 * D)
        nc.sync.dma_start(wk_sb[:, wsl], wk[:, wsl])
        wvt = big.tile([128, 8 * D], BF16, tag="wo", name=f"wv{part}")
        nc.sync.dma_start(wvt[:], wv[:, wsl])
        wv_tiles.append(wvt)
        wqt = wstr.tile([128, 8 * D], BF16, tag="w1", name=f"wq0_{part}")
        nc.sync.dma_start(wqt[:], wq[:, wsl])
        wq0_tiles.append(wqt)
        if part == 0:
            for g, (c0, w) in enumerate([(0, 2), (2, 2), (4, 4)]):
                t = hspool.tile([128, w, S], BF16, tag=f"hsg{g}", name=f"hsg{g}")
                nc.sync.dma_start(t[:], hsT_v[:, c0:c0 + w, :])
                for c in range(w):
                    hs_tiles[c0 + c] = (t, c)
        else:
            t = hspool.tile([128, 8, S], BF16, tag=f"hs{part}", name=f"hs{part}")
            nc.sync.dma_start(t[:], hsT_v[:, csl, :])
            for c in range(8):
                hs_tiles[part * 8 + c] = (t, c)
            if part == 3:
                load_consts()

    def hs_chunk(cc, sl):
        t, c = hs_tiles[cc]
        return t[:, c, sl]

    def mm_pair(outs, lhsT, rhss, start, stop):
        """Consecutive matmuls sharing one stationary operand: elide the
        redundant LDWEIGHTS on all but the first."""
        for i, (o, r) in enumerate(zip(outs, rhss)):
            inst = nc.tensor.matmul(o, lhsT, r, start=start, stop=stop)
            if i > 0:
                inst.ins.ldweights = False

    w_srcs = {j: wq[:, j * HID:(j + 1) * HID] for j in range(NQ)}

    def project(src_key, dst_sb):
        """dst_sb[128, S] (bf16) = (W_col^T @ hs) for one 128-wide column."""
        ps = [psum.tile([128, SC], F32, tag="mm512", name=f"pj{s}")
              for s in range(NSC)]
        for half in range(8):
            # alternate between the two rings -> effective prefetch depth 4
            pl, tg = (wstr, "w1") if half % 2 == 0 else (big, "wo")
            w_t = pl.tile([128, 8 * D], BF16, tag=tg)
            nc.sync.dma_start(
                w_t[:], w_srcs[src_key][:, half * 8 * D:(half + 1) * 8 * D])
            for c in range(NCH // 8):
                cc = half * 8 + c
                mm_pair([ps[s][:] for s in range(NSC)],
                        w_t[:, c * D:(c + 1) * D],
                        [hs_chunk(cc, slice(s * SC, (s + 1) * SC))
                         for s in range(NSC)],
                        start=(cc == 0), stop=(cc == NCH - 1))
        for s in range(NSC):
            # vector, not scalar: the scalar queue is ~13us deep with the
            # exp chain, and rope + next-head scores wait on this copy
            nc.vector.tensor_copy(dst_sb[:, s * SC:(s + 1) * SC], ps[s][:])

    def rope(src_sb):
        # in-place: src = src * cosT + (perm.T @ src) * sinT2
        shs = [psum_tr.tile([128, SC], F32, tag="shift", name=f"sh{s}")
               for s in range(NSC)]
        mm_pair([sh[:] for sh in shs], perm_sb[:],
                [src_sb[:, s * SC:(s + 1) * SC] for s in range(NSC)],
                start=True, stop=True)
        for s in range(NSC):
            sl = slice(s * SC, (s + 1) * SC)
            tmp = small.tile([128, SC], F32, tag="tanh")
            # cos-multiply first: it does not depend on the perm matmul, so
            # it drains from the DVE queue while the matmul is in flight
            nc.vector.tensor_mul(src_sb[:, sl], src_sb[:, sl], cos_sb[:, sl])
            nc.vector.tensor_mul(tmp[:], shs[s][:], sin_sb[:, sl])
            nc.vector.tensor_add(src_sb[:, sl], src_sb[:, sl], tmp[:])

    # Causal geometry: key tile t2 only attends queries s1 >= t2*128, so its
    # score/exp/ov tile within chunk ch is only w = 512 - max(0, t2*128 -
    # ch*512) columns wide (right-aligned). Diagonal 128-blocks (first 128
    # cols of each ragged tile in its own chunk) take a triangular mask.
    def tile_geom(ch):
        out = []
        for t2 in range(0, min(NQ, (ch + 1) * 4)):
            w = SC - max(0, t2 * D - ch * SC)
            out.append((t2, w, SC - w))
        return out

    EOFFS, _pos = {}, 0
    for _ch in range(NSC):
        for _t2, _w, _off in tile_geom(_ch):
            EOFFS[(_t2, _ch)] = (_pos, _w)
            _pos += _w
    EXPW = _pos  # 4608

    def scores_exp(j, qrope):
        # per chunk: score matmuls -> tanh/exp -> mask, then immediately the
        # chunk's denominator DVE tree, so the ch0 tree is not queued behind
        # the ch1 exp chain on the vector engine
        expT = big.tile([128, EXPW], BF16, tag="big8k", bufs=1,
                        name=f"expT{j}")
        accbs = []
        for ch in range(NSC):
            geom = tile_geom(ch)
            for t2, w, off in geom:
                sc_ps = psum.tile([128, SC], F32, tag="mm512", name="sc")
                nc.tensor.matmul(sc_ps[:, 0:w], k_sb[:, t2 * D:(t2 + 1) * D],
                                 qrope[:, ch * SC + off:(ch + 1) * SC],
                                 start=True, stop=True)
                tmp = small.tile([128, SC], F32, tag="tanh")
                nc.scalar.activation(tmp[:, 0:w], sc_ps[:, 0:w], AF.Tanh,
                                     scale=SCALE / LOGIT_CAP)
                es, _ = EOFFS[(t2, ch)]
                dst = expT[:, es:es + w]
                nc.scalar.activation(dst, tmp[:, 0:w], AF.Exp, scale=LOGIT_CAP)
                if t2 // 4 == ch:
                    nc.vector.tensor_mul(expT[:, es:es + D], expT[:, es:es + D],
                                         mask_sb[:])
            acc = small.tile([128, SC], F32, tag="tanh", name="dacc")
            es0, w0 = EOFFS[(geom[0][0], ch)]
            nc.vector.tensor_copy(acc[:], expT[:, es0:es0 + w0])
            for t2, w, off in geom[1:]:
                es, _ = EOFFS[(t2, ch)]
                nc.vector.tensor_add(acc[:, off:SC], acc[:, off:SC],
                                     expT[:, es:es + w])
            accb = small.tile([128, SC], BF16, tag="acb", name=f"accb{ch}")
            nc.vector.tensor_copy(accb[:], acc[:])
            accbs.append(accb)
        return expT, accbs

    def attn_finish(j, expT, accbs):
        for ch in range(NSC):
            sl = slice(ch * SC, (ch + 1) * SC)
            geom = tile_geom(ch)
            # ov first: its matmuls chase the exp chain tile by tile
            ov = psum.tile([128, SC], F32, tag="mm512", name="ovps")
            for i, (t2, w, off) in enumerate(geom):
                es, _ = EOFFS[(t2, ch)]
                nc.tensor.matmul(ov[:, off:SC], v_sb[:, t2, :],
                                 expT[:, es:es + w],
                                 start=(i == 0), stop=(i == len(geom) - 1),
                                 skip_group_check=True)
            dn = psum_dn.tile([1, SC], F32, tag="dn")
            nc.tensor.matmul(dn[:], ones_sb[:], accbs[ch][:],
                             start=True, stop=True)
            # reciprocal on the [1,512] row, then broadcast in bf16 (an fp32
            # broadcast matmul is 8x slower and trips the HAM throttle; a
            # gpsimd partition_broadcast was tried and is ~1us + sem hops,
            # stalling the DVE queue at the final multiply)
            dnr = small.tile([1, SC], F32, tag="rcx", name="dnr")
            nc.vector.reciprocal_approx_fast(out=dnr[:], in_=dn[:])
            dnrb = small.tile([1, SC], BF16, tag="rcx", name="dnrb")
            nc.scalar.copy(dnrb[:], dnr[:])
            rcb_ps = psum.tile([128, SC], F32, tag="mm512", name="rcbps")
            nc.tensor.matmul(rcb_ps[:], onesr_sb[:], dnrb[:],
                             start=True, stop=True)
            rcbs = small.tile([128, SC], BF16, tag="rcx", name="rcbs")
            nc.scalar.copy(rcbs[:], rcb_ps[:])
            nc.vector.tensor_mul(oT_sb[:, j, sl], ov[:], rcbs[:])

    # ---- fused K+V+q0 projection chasing the hs stream --------------------
    vT_sb = qpool.tile([128, S], BF16, tag="qh", name="vT")
    qrope = qpool.tile([128, S], BF16, tag="qh", name="q0")
    kps = [psum.tile([128, SC], F32, tag="mm512", name=f"kp{s}")
           for s in range(NSC)]
    vps = [psum.tile([128, SC], F32, tag="mm512", name=f"vp{s}")
           for s in range(NSC)]
    qps = [psum_tr.tile([128, SC], F32, tag="shift", name=f"qp{s}")
           for s in range(NSC)]
    for cc in range(NCH):
        part, c = cc >> 3, cc & 7
        hsp = [hs_chunk(cc, slice(s * SC, (s + 1) * SC)) for s in range(NSC)]
        st, sp = (cc == 0), (cc == NCH - 1)
        mm_pair([p[:] for p in kps], wk_sb[:, cc * D:(cc + 1) * D], hsp,
                start=st, stop=sp)
        mm_pair([p[:] for p in vps], wv_tiles[part][:, c * D:(c + 1) * D], hsp,
                start=st, stop=sp)
        mm_pair([p[:] for p in qps], wq0_tiles[part][:, c * D:(c + 1) * D], hsp,
                start=st, stop=sp)
    for s in range(NSC):
        sl = slice(s * SC, (s + 1) * SC)
        nc.scalar.copy(k_sb[:, sl], kps[s][:])
        nc.vector.tensor_copy(vT_sb[:, sl], vps[s][:])
        nc.vector.tensor_copy(qrope[:, sl], qps[s][:])
    rope(k_sb)
    for t2 in range(NQ):
        vt = psum_tr.tile([128, SC], BF16, tag="shift", name=f"vt{t2}")
        nc.tensor.transpose(vt[:, :D], vT_sb[:, t2 * D:(t2 + 1) * D],
                            ident_sb[:])
        nc.vector.tensor_copy(v_sb[:, t2, :], vt[:, :D])
    rope(qrope)

    # ---- output projection helpers ----------------------------------------
    # Wo chunks stream through the (dead after K projection) wk_sb buffer as
    # an 8-deep ring; DMA issues run ~3 chunks ahead of the matmuls so loads
    # queue before the (latency-tolerant) output stores on the FIFO queue.
    _wo_issued = set()

    def issue_wo(ec):
        if ec < NCH and ec not in _wo_issued:
            _wo_issued.add(ec)
            base = (ec % 8) * QW
            nc.sync.dma_start(wk_sb[:, base:base + QW],
                              wo[:, ec * QW:(ec + 1) * QW])

    def oproj_open(ec, pl, tag):
        issue_wo(ec)
        base = (ec % 8) * QW
        wo_t = wk_sb[:, base:base + QW]
        ps = [pl.tile([128, SC], F32, tag=tag, name=f"op{ec}_{s}")
              for s in range(NSC)]
        for hh in range(NQ - 1):
            for s in range(NSC):
                nc.tensor.matmul(ps[s][:], wo_t[:, hh * D:(hh + 1) * D],
                                 oT_sb[:, hh, s * SC:(s + 1) * SC],
                                 start=(hh == 0), stop=False)
        return wo_t, ps

    def oproj_close(ec, wo_t, ps):
        for s in range(NSC):
            nc.tensor.matmul(ps[s][:], wo_t[:, (NQ - 1) * D:NQ * D],
                             oT_sb[:, NQ - 1, s * SC:(s + 1) * SC],
                             start=False, stop=True)
        for s in range(NSC):
            ot = small.tile([128, SC], BF16, tag="rcx", name="ot")
            nc.vector.tensor_copy(ot[:], ps[s][:])
            nc.sync.dma_start(
                outp[ec * 128:(ec + 1) * 128, s * SC:(s + 1) * SC], ot[:])

    # ---- fused Q projection + attention, software-pipelined ---------------
    # Emit head j+1's projection between head j's scores and its softmax
    # finish, so the in-order tensor engine never waits on the scalar/vector
    # exp chain. The last head overlaps its softmax with the first two
    # O-projection chunks' hh=0..6 accumulation instead.
    pre = []
    for j in range(NQ):
        expT, accbs = scores_exp(j, qrope)
        if j + 1 < NQ:
            qrope_next = qpool.tile([128, S], BF16, tag="qh", name=f"q{j + 1}")
            project(j + 1, qrope_next)
        else:
            pre.append((0, *oproj_open(0, psum_tr, "shift")))
        attn_finish(j, expT, accbs)
        if j + 1 < NQ:
            rope(qrope_next)
            qrope = qrope_next

    # ---- output projection: out[e, s] = sum_hh wo[:, hh, e]^T @ oT[:, hh, s]
    for ec, wo_t, ps in pre:
        for k in range(1, 4):
            issue_wo(k)
        oproj_close(ec, wo_t, ps)
    for ec in range(1, NCH):
        issue_wo(ec + 3)
        wo_t, ps = oproj_open(ec, psum, "mm512")
        oproj_close(ec, wo_t, ps)


# --------------------------------------------------------------------------
# host side
# --------------------------------------------------------------------------

def _rope_tables(position_ids):
    pos = np.asarray(position_ids).reshape(-1).astype(np.int64)
    inv_freq = (1.0 / (ROPE_THETA ** (np.arange(0, D, 2, dtype=np.float32) / D))
                ).astype(np.float32)
    t = np.arange(S, dtype=np.float32)
    freqs = np.outer(t, inv_freq).astype(np.float32)       # (S, D/2)
    emb = np.concatenate((freqs, freqs), axis=-1)          # (S, D)
    cos = np.cos(emb).astype(np.float32)[pos]              # (S, D)
    sin = np.sin(emb).astype(np.float32)[pos]
    cosT = np.ascontiguousarray(cos.T)                     # (D, S)
    sinT = np.ascontiguousarray(sin.T)
    sinT2 = sinT.copy()
    sinT2[: D // 2] *= -1.0                                # rotate_half sign
    return cosT, sinT2


def _mask_patterns(attention_mask):
    # single diagonal-block pattern: allowed(s2 = p, s1 = j) for p, j < 128
    am = np.asarray(attention_mask)[0, 0]                  # (S_q, S_k)
    return np.ascontiguousarray((am[:D, :D].T > -0.5).astype(np.float32)).astype(BF)


_NC = None


def _get_nc():
    global _NC
    if _NC is None:
        _NC = build_nc()
    return _NC


def make_in_maps(hidden_states, Wq, Wk, Wv, Wo, attention_mask, position_ids):
    hsT = np.ascontiguousarray(
        np.asarray(hidden_states)[0].T.astype(np.float32)).astype(BF)
    cosT, sinT2 = _rope_tables(position_ids)
    masks = _mask_patterns(attention_mask)
    perm = np.zeros((D, D), dtype=np.float32)
    for d in range(D):
        perm[(d + 64) % 128, d] = 1.0
    perm = perm.astype(BF)
    ident = np.eye(D, dtype=np.float32).astype(BF)
    onesd = np.ones((D, 1), dtype=np.float32).astype(BF)
    Wq = np.asarray(Wq)
    Wk = np.asarray(Wk)
    Wv = np.asarray(Wv)
    Wo = np.asarray(Wo)
    in_maps = []
    for c in range(NCORES):
        # Wo rows of this core: [QW, HID] -> [hh, d, e] -> [d, e_chunk*hh*128]
        woc = Wo[c * QW:(c + 1) * QW, :].reshape(NQ, D, HID)
        woc = np.ascontiguousarray(
            woc.transpose(1, 2, 0).reshape(D, NCH, 128, NQ).transpose(0, 1, 3, 2)
        ).reshape(D, NCH * QW)
        # wq [8192, 1024] -> [p, head*chunk*m]; wk/wv [8192, 128] -> [p, chunk*m]
        wqc = np.ascontiguousarray(
            Wq[:, c * QW:(c + 1) * QW].reshape(NCH, D, NQ, D)
            .transpose(1, 2, 0, 3)).reshape(D, NQ * HID)
        wkc = np.ascontiguousarray(
            Wk[:, c * D:(c + 1) * D].reshape(NCH, D, D)
            .transpose(1, 0, 2)).reshape(D, HID)
        wvc = np.ascontiguousarray(
            Wv[:, c * D:(c + 1) * D].reshape(NCH, D, D)
            .transpose(1, 0, 2)).reshape(D, HID)
        in_maps.append({
            "hsT": hsT,
            "wq": wqc.astype(BF),
            "wk": wkc.astype(BF),
            "wv": wvc.astype(BF),
            "wo": woc.astype(BF),
            "cosT": cosT.astype(BF), "sinT2": sinT2.astype(BF), "masks": masks,
            "perm": perm, "ident": ident, "onesd": onesd,
            "onesr": np.ones((1, D), dtype=np.float32).astype(BF),
        })
    return in_maps


def kernel(hidden_states, Wq, Wk, Wv, Wo, attention_mask, position_ids,
           _trace=False):
    nc = _get_nc()
    in_maps = make_in_maps(hidden_states, Wq, Wk, Wv, Wo, attention_mask,
                           position_ids)
    res = run_bass_kernel_spmd(nc, in_maps, list(range(NCORES)), trace=_trace)
    out = np.zeros((HID, S), dtype=np.float32)
    for c in range(NCORES):
        out += res.results[c]["outp"].astype(np.float32)
    ret = np.ascontiguousarray(out.T).reshape(B, S, HID)
    if _trace:
        kernel.last_exec_time_ns = res.exec_time_ns
        kernel.last_results = res
    return ret
